# revision 14
# baseline (speedup 1.0000x reference)
"""Trainium2 Bass kernel for nn_EdgeEncoder (moe_routing).

Strategy
--------
Each of E edges is routed to 1 of 9 expert MLPs (4 -> 256 -> 256), then
  out = relu(concat([type_embed[tid], source_embed[sid], pv]) @ Wf + bf).

Host (numpy, cheap O(E) work):
  * scale/mask params, group edge indices by expert (base type) at
    256-edge granularity, split evenly over 8 cores (identical layout on
    every core so one SPMD program serves all 8),
  * algebraic fusions: b1 rides a ones-row inside layer 1;
    V[t] = W2[t] @ Wf_pv fuses layer 2 with the final projection;
    G[t] = [const; type_embed@Wf_t; source_embed@Wf_s] turns the
    embedding gathers + all biases into one small matmul against the
    one-hot rows.

Device, all bf16 operands (fp32 PSUM accumulate; rel-err gate is 2e-2):
  The per-512-edge block needs h = relu(W1e.T @ x1) (2 matmuls, K=5),
  the G part (2 matmuls, K=24) and V part (4 matmuls, K=128) of the
  output. The four small-K matmuls are packed into ONE matmul slot via
  tile_position row-strips (0/32/64/96): the host ships a [32, L] input
  (x+ones+one-hots), the device replicates it to all four 32-partition
  strips with SBUF->SBUF DMA, and the four strip matmuls run
  concurrently in the PE array. Per block: 1 packed slot + 4 V matmuls.
  relu-h is one [128,1024] ACT op, out-relu one [128,1024] DVE op
  (PSUM->SBUF, the only engines that can read PSUM). Outputs are stored
  bf16 in a DMA-native packed layout and unscrambled on host.
"""

import math
import os

import ml_dtypes
import numpy as np

import concourse.bacc as bacc
import concourse.bass as bass
import concourse.mybir as mybir
import concourse.tile as tile
from concourse.bass_utils import run_bass_kernel_spmd

# ---- static module configuration (mirrors the torch source) ----
T = 9            # base types ("experts")
P_MAX = 4
D = 256
N_TYPES = 14
N_SRC = 5
NCORES = 8
BLOCK = 512      # edges per block (one PSUM bank per 128-out-dims half)
GRAN = 1         # run granularity (expert segments padded to per-core exact)

BASE_MAP = np.array([0, 0, 0, 1, 1, 1, 2, 2, 3, 4, 5, 6, 7, 8], dtype=np.int32)
PCOUNT = np.array([2, 2, 1, 1, 1, 1, 3, 2, 4], dtype=np.int32)
SCALES = np.ones((T, P_MAX), dtype=np.float32)
SCALES[0, :2] = [1.0, 1e-06]      # nmos  m, w
SCALES[1, :2] = [1.0, 1e-06]      # pmos  m, w
SCALES[2, 0] = 1.0                # balun rout
SCALES[3, 0] = 1000.0             # resistor r
SCALES[4, 0] = 1e-12              # capacitor c
SCALES[5, 0] = 1e-09              # inductor l
SCALES[6, :3] = [1.0, 1.0, 1.0]   # vsource dc, mag, phase
SCALES[7, :2] = [0.001, 0.001]    # isource dc, mag
SCALES[8, :4] = [1.0, 1.0, 1e9, 1.0]  # port dbm, dc, freq, num

# xu strip layout (replicated at partition offsets 0/32/64/96):
#   rows 0-3: scaled params, row 4: ones (valid), rows 5-18: type one-hot,
#   rows 19-23: source one-hot, rows 24-31: zero
K_L1 = 5                   # x rows + ones
K_G = 24                   # rows 0..23 (x rows are zero in the G weights)
STRIPS = (0, 32, 64, 96)   # (L1 h0, L1 h1, G g0, G g1)

_F32 = mybir.dt.float32
_BF16 = mybir.dt.bfloat16
_WARM_BURST = int(os.environ.get("EDGEENC_WARM_BURST", "10"))

_PROGRAM_CACHE: dict = {}
LAST_RESULT = None  # BassKernelResults of the most recent run (for test harness)


def _layout(base_ids: np.ndarray):
    """Per-expert per-core segment sizes (multiples of GRAN), identical on
    every core so one program serves all 8."""
    n_t = np.bincount(base_ids, minlength=T)
    m_t = np.zeros(T, dtype=np.int64)
    for t in range(T):
        if n_t[t] > 0:
            per_core = math.ceil(n_t[t] / NCORES)
            m_t[t] = math.ceil(per_core / GRAN) * GRAN
    L0 = int(m_t.sum())
    L = math.ceil(L0 / BLOCK) * BLOCK
    # fold the tail pad into the last present expert's segment
    last = int(np.nonzero(m_t)[0][-1])
    m_t[last] += L - L0
    return n_t, m_t, L


def _build_order(base_ids: np.ndarray, n_t, m_t, L) -> np.ndarray:
    """ORD[c, j] = global edge index at per-core slot j (or -1 = pad)."""
    ORD = np.full((NCORES, L), -1, dtype=np.int64)
    off = 0
    for t in range(T):
        if m_t[t] == 0:
            continue
        seg = int(m_t[t])
        idx = np.nonzero(base_ids == t)[0]
        arr = np.full(NCORES * seg, -1, dtype=np.int64)
        arr[: idx.shape[0]] = idx
        ORD[:, off : off + seg] = arr.reshape(NCORES, seg)
        off += seg
    return ORD


def _host_inputs(type_ids, source_ids, params, ORD):
    """XU[c] = [32, L] bf16: x rows, ones, type one-hot, src one-hot, zeros."""
    base_ids = BASE_MAP[type_ids]
    scales = SCALES[base_ids]                                  # [E,4]
    validp = np.arange(P_MAX)[None, :] < PCOUNT[base_ids][:, None]
    x = np.where(validp, params.astype(np.float32) / scales, 0.0).astype(np.float32)

    L = ORD.shape[1]
    XU = np.zeros((NCORES, 32, L), dtype=np.float32)
    valid = ORD >= 0
    ids = ORD[valid]
    tmp = np.zeros((NCORES, L, P_MAX), dtype=np.float32)
    tmp[valid] = x[ids]
    XU[:, 0:P_MAX, :] = tmp.transpose(0, 2, 1)
    XU[:, P_MAX, :] = valid
    ci, co = np.nonzero(valid)
    XU[ci, 5 + type_ids[ids], co] = 1.0
    XU[ci, 19 + source_ids[ids], co] = 1.0
    return XU.astype(ml_dtypes.bfloat16)


def _host_weights(type_embed, source_embed, W1, b1, W2, b2, Wf, bf):
    f = np.float32
    W1 = W1.astype(f); b1 = b1.astype(f); W2 = W2.astype(np.float64)
    b2 = b2.astype(f); Wf = Wf.astype(f); bf = bf.astype(f)
    type_embed = type_embed.astype(f); source_embed = source_embed.astype(f)

    Wft, Wfs, Wfp = Wf[:D], Wf[D : 2 * D], Wf[2 * D :]
    # V[t] = W2[t] @ Wf_pv (f64), fusing layer 2 with the final projection.
    V = (W2 @ Wfp.astype(np.float64)).astype(f)                 # [9,256,256]
    gt = type_embed @ Wft                                       # [14,256]
    gs = source_embed @ Wfs                                     # [5,256]
    gc = b2 @ Wfp + bf[None, :]                                 # [9,256]

    # W4 [128, T*512]: per expert t the four strip lhsT blocks
    #   [h0 | h1 | g0 | g1], each 128 cols.
    W4 = np.zeros((128, T * D * 2), dtype=f)
    VR = np.zeros((128, T * D * 2), dtype=f)
    for t in range(T):
        c = t * 512
        for h in range(2):
            W4[0:4, c + h * 128 : c + (h + 1) * 128] = W1[t][:, h * 128 : (h + 1) * 128]
            W4[4, c + h * 128 : c + (h + 1) * 128] = b1[t][h * 128 : (h + 1) * 128]
        for g in range(2):
            cg = c + 256 + g * 128
            W4[64 + 4, cg : cg + 128] = gc[t][g * 128 : (g + 1) * 128]
            W4[64 + 5 : 64 + 19, cg : cg + 128] = gt[:, g * 128 : (g + 1) * 128]
            W4[64 + 19 : 64 + 24, cg : cg + 128] = gs[:, g * 128 : (g + 1) * 128]
            # replicate for the 96-strip (g1 reads partitions 96..119)
            W4[96 + 4 : 96 + 24, cg : cg + 128] = W4[64 + 4 : 64 + 24, cg : cg + 128]
        # V chunks (h, g): rows = h dims, cols = out dims
        for h in range(2):
            for g in range(2):
                VR[:, c + h * 256 + g * 128 : c + h * 256 + (g + 1) * 128] = (
                    V[t][h * 128 : (h + 1) * 128, g * 128 : (g + 1) * 128])
    # strips 32/96 read their own partition rows; copy h1/g1 blocks there
    for t in range(T):
        c = t * 512
        W4[32:37, c + 128 : c + 256] = W4[0:5, c + 128 : c + 256]
        W4[0:5, c + 128 : c + 256] = 0.0
        W4[64 + 4 : 64 + 24, c + 384 : c + 512] = 0.0
    return W4.astype(ml_dtypes.bfloat16), VR.astype(ml_dtypes.bfloat16)


def _block_runs(m_t, L):
    """Per block: list of (c0, c1, expert) with cols relative to the block."""
    bounds = []
    off = 0
    for t in range(T):
        if m_t[t]:
            bounds.append((off, off + int(m_t[t]), t))
            off += int(m_t[t])
    if off < L:  # tail pad rides with the last expert
        bounds[-1] = (bounds[-1][0], L, bounds[-1][2])
    NB = L // BLOCK
    runs = [[] for _ in range(NB)]
    for (s0, s1, t) in bounds:
        b0, b1 = s0 // BLOCK, (s1 - 1) // BLOCK
        for b in range(b0, b1 + 1):
            c0 = max(s0 - b * BLOCK, 0)
            c1 = min(s1 - b * BLOCK, BLOCK)
            runs[b].append((c0, c1, t))
    return runs


def _build_program(m_t: tuple, L: int):
    """One compiled SPMD program for the given segment layout.

    Software-pipelined one block deep: emit slot(b+1) before V(b) so the
    PE never waits on the relu-h chain. PSUM: h pool 2x[128,512] (2
    banks), out pool 3x[128,1024] (6 banks) — the 3-deep out rotation
    keeps the G(b) -> V(b) -> DVE(b) -> G(b+3) buffer-reuse cycle off
    the critical path.
    """
    key = (m_t, L, _WARM_BURST)
    if key in _PROGRAM_CACHE:
        return _PROGRAM_CACHE[key]

    NB = L // BLOCK
    NSB = (NB + 1) // 2
    runs = _block_runs(np.asarray(m_t, dtype=np.int64), L)

    nc = bacc.Bacc("TRN2", target_bir_lowering=False, debug=False,
                   num_devices=NCORES)
    xu_d = nc.dram_tensor("xu", [32, L], _BF16, kind="ExternalInput")
    xu0_d = nc.dram_tensor("xu0", [128, 2 * BLOCK], _BF16, kind="ExternalInput")
    w4_d = nc.dram_tensor("w4", [128, T * 512], _BF16, kind="ExternalInput")
    vr_d = nc.dram_tensor("vr", [128, T * 512], _BF16, kind="ExternalInput")
    out_d = nc.dram_tensor("out", [128, NB * 1024], _BF16, kind="ExternalOutput")

    RELU = mybir.ActivationFunctionType.Relu

    with tile.TileContext(nc) as tc:
        with (
            tc.tile_pool(name="wts", bufs=1) as wts,
            tc.tile_pool(name="inp", bufs=3) as inp,
            tc.tile_pool(name="hsb", bufs=2) as hsbp,
            tc.tile_pool(name="osb", bufs=2) as osbp,
            tc.tile_pool(name="hps", bufs=3, space=bass.MemorySpace.PSUM) as hps,
            tc.tile_pool(name="ops", bufs=5, space=bass.MemorySpace.PSUM) as ops,
        ):
            w4 = wts.tile([128, T * 512], _BF16)
            vr = wts.tile([128, T * 512], _BF16)

            # prime the ACT table (Relu) before the first real activation
            prime = wts.tile([1, 8], _BF16)
            nc.gpsimd.memset(prime[:], 0.0)
            nc.scalar.activation(prime[0:1, 0:4], prime[0:1, 4:8], RELU)

            # bf16 warm-up burst: raise the PE HAM clock gate while the
            # first input/weight DMAs land
            if _WARM_BURST:
                wmw = wts.tile([128, 128], _BF16)
                wma = wts.tile([128, BLOCK], _BF16)
                nc.gpsimd.memset(wmw[:], 0.0)
                nc.gpsimd.memset(wma[:], 0.0)
                wmp = hps.tile([128, BLOCK], _F32, name="warmps", tag="h")
                for i in range(_WARM_BURST):
                    nc.tensor.matmul(wmp[:], wmw[:], wma[:],
                                     start=True, stop=True)

            emitted_w = set()

            def emit_weights(b):
                for (c0, c1, t) in runs[b]:
                    if t not in emitted_w:
                        emitted_w.add(t)
                        nc.sync.dma_start(w4[:, t * 512 : (t + 1) * 512],
                                          w4_d.ap()[:, t * 512 : (t + 1) * 512])
                        nc.sync.dma_start(vr[:, t * 512 : (t + 1) * 512],
                                          vr_d.ap()[:, t * 512 : (t + 1) * 512])

            xuts = {}

            def emit_input_sb(sb):
                if sb >= NSB:
                    return
                sbw = min(2 * BLOCK, L - sb * 1024)
                xut = inp.tile([128, 2 * BLOCK], _BF16, name=f"xu{sb}", tag="xu")
                if sb == 0:
                    # host pre-replicated: one DMA, no SBUF->SBUF chain at start
                    nc.gpsimd.dma_start(xut[:, 0:sbw], xu0_d.ap()[:, 0:sbw])
                else:
                    nc.gpsimd.dma_start(xut[0:32, 0:sbw],
                                        xu_d.ap()[:, sb * 1024 : sb * 1024 + sbw])
                    nc.gpsimd.dma_start(xut[32:64, 0:sbw], xut[0:32, 0:sbw])
                    nc.gpsimd.dma_start(xut[64:128, 0:sbw], xut[0:64, 0:sbw])
                xuts[sb] = xut

            hpt = {}   # (b, h) -> [128,512] psum tile
            hst = {}   # b -> [128,1024] sbuf bf16 relu(h)
            opt = {}   # b -> [128,1024] psum out accumulator
            ost = {}   # sb -> [128,2048] sbuf bf16 out staging

            def emit_slot(b):
                """L1 h0/h1 + G g0/g1 on 4 concurrent row strips."""
                if b >= NB:
                    return
                emit_weights(b)
                if b % 2 == 0:
                    emit_input_sb(b // 2 + 1)   # prefetch next superblock
                xut = xuts[b // 2]
                off = (b % 2) * BLOCK
                hpt[(b, 0)] = hps.tile([128, BLOCK], _F32, name=f"h{b}_0", tag="h")
                hpt[(b, 1)] = hps.tile([128, BLOCK], _F32, name=f"h{b}_1", tag="h")
                opt[(b, 0)] = ops.tile([128, BLOCK], _F32, name=f"o{b}_0", tag="o")
                opt[(b, 1)] = ops.tile([128, BLOCK], _F32, name=f"o{b}_1", tag="o")
                started = set()
                for wi in range(4):
                    s = STRIPS[wi]
                    k = K_L1 if wi < 2 else K_G
                    for (c0, c1, t) in runs[b]:
                        wcol = t * 512 + wi * 128
                        if wi < 2:
                            dst = hpt[(b, wi)][:, c0:c1]
                            start = True
                        else:
                            g = wi - 2
                            dst = opt[(b, g)][:, c0:c1]
                            start = g not in started  # first write to bank
                            started.add(g)
                        nc.tensor.matmul(
                            dst,
                            w4[s : s + k, wcol : wcol + 128],
                            xut[s : s + k, off + c0 : off + c1],
                            start=start, stop=(wi < 2),
                            tile_position=(s, 0),
                        )
                # relu h: 2 ACT ops (frees the h banks quickly)
                hs = hsbp.tile([128, 1024], _BF16, name=f"hs{b}", tag="hs")
                nc.scalar.activation(hs[:, 0:BLOCK], hpt[(b, 0)][:], RELU)
                nc.scalar.activation(hs[:, BLOCK:1024], hpt[(b, 1)][:], RELU)
                hst[b] = hs

            def emit_v_and_out(b):
                """V accumulation (4 chunks) + DVE out-relu + out DMA."""
                last = {}
                for h in range(2):
                    for g in range(2):
                        for (c0, c1, t) in runs[b]:
                            last[g] = (h, g, c0)
                sb = b // 2
                if b % 2 == 0:
                    ost[sb] = osbp.tile([128, 2048], _BF16, name=f"os{sb}",
                                        tag="os")
                for h in range(2):
                    for g in range(2):
                        for (c0, c1, t) in runs[b]:
                            vcol = t * 512 + h * 256 + g * 128
                            nc.tensor.matmul(
                                opt[(b, g)][:, c0:c1],
                                vr[:, vcol : vcol + 128],
                                hst[b][:, h * BLOCK + c0 : h * BLOCK + c1],
                                start=False, stop=(last[g] == (h, g, c0)),
                            )
                for g in range(2):
                    nc.vector.tensor_scalar_max(
                        ost[sb][:, (b % 2) * 1024 + g * BLOCK
                                : (b % 2) * 1024 + (g + 1) * BLOCK],
                        opt[(b, g)][:], 0.0)
                sbw = min(2 * BLOCK, L - sb * 1024)
                if b == 2 * sb + 1 or b == NB - 1:
                    nc.sync.dma_start(
                        out_d.ap()[:, sb * 2048 : sb * 2048 + 2 * sbw],
                        ost[sb][:, 0 : 2 * sbw])

            emit_input_sb(0)
            emit_slot(0)
            for b in range(NB):
                emit_slot(b + 1)
                emit_v_and_out(b)

    nc.compile()
    _PROGRAM_CACHE[key] = nc
    return nc


def kernel(type_ids, source_ids, params, type_embed, source_embed,
           W1, b1, W2, b2, Wf, bf):
    global LAST_RESULT
    type_ids = np.asarray(type_ids, dtype=np.int32)
    source_ids = np.asarray(source_ids, dtype=np.int32)
    params = np.asarray(params, dtype=np.float32)
    E = type_ids.shape[0]

    base_ids = BASE_MAP[type_ids]
    n_t, m_t, L = _layout(base_ids)
    ORD = _build_order(base_ids, n_t, m_t, L)
    XU = _host_inputs(type_ids, source_ids, params, ORD)
    W4, VR = _host_weights(
        np.asarray(type_embed), np.asarray(source_embed),
        np.asarray(W1), np.asarray(b1), np.asarray(W2), np.asarray(b2),
        np.asarray(Wf), np.asarray(bf))

    nc = _build_program(tuple(int(v) for v in m_t), L)

    sbw0 = min(1024, L)
    in_maps = []
    for c in range(NCORES):
        xu0 = np.zeros((128, 1024), dtype=ml_dtypes.bfloat16)
        xu0[:, :sbw0] = np.tile(XU[c][:, :sbw0], (4, 1))
        in_maps.append({"xu": np.ascontiguousarray(XU[c]), "xu0": xu0,
                        "w4": W4, "vr": VR})

    trace = bool(int(os.environ.get("EDGEENC_TRACE", "0")))
    res = run_bass_kernel_spmd(nc, in_maps, core_ids=list(range(NCORES)),
                               trace=trace)
    LAST_RESULT = res

    NB = L // BLOCK
    full = np.zeros((E, D), dtype=np.float32)
    for c in range(NCORES):
        oc = res.results[c]["out"]                     # [128, NB*1024] bf16
        oc = np.asarray(oc)
        if oc.dtype != np.float32:
            oc = oc.astype(np.float32)
        # cols: [block b][g half][512 edges] -> [D, L]
        oc = oc.reshape(128, NB, 2, BLOCK)             # p, b, g, e
        oc = oc.transpose(2, 0, 1, 3).reshape(D, L)    # d = g*128+p
        sel = ORD[c] >= 0
        full[ORD[c][sel]] = np.ascontiguousarray(oc[:, sel].T)
    return full


# revision 16
# speedup vs baseline: 1.1009x; 1.1009x over previous
"""Trainium2 Bass kernel for nn_EdgeEncoder (moe_routing).

Strategy
--------
Each of E edges is routed to 1 of 9 expert MLPs (4 -> 256 -> 256), then
  out = relu(concat([type_embed[tid], source_embed[sid], pv]) @ Wf + bf).

Host (numpy, cheap O(E) work):
  * scale/mask params, group edge indices by expert (base type) at
    256-edge granularity, split evenly over 8 cores (identical layout on
    every core so one SPMD program serves all 8),
  * algebraic fusions: b1 rides a ones-row inside layer 1;
    V[t] = W2[t] @ Wf_pv fuses layer 2 with the final projection;
    G[t] = [const; type_embed@Wf_t; source_embed@Wf_s] turns the
    embedding gathers + all biases into one small matmul against the
    one-hot rows.

Device, all bf16 operands (fp32 PSUM accumulate; rel-err gate is 2e-2):
  The per-512-edge block needs h = relu(W1e.T @ x1) (2 matmuls, K=5),
  the G part (2 matmuls, K=24) and V part (4 matmuls, K=128) of the
  output. The four small-K matmuls are packed into ONE matmul slot via
  tile_position row-strips (0/32/64/96): the host ships a [32, L] input
  (x+ones+one-hots), the device replicates it to all four 32-partition
  strips with SBUF->SBUF DMA, and the four strip matmuls run
  concurrently in the PE array. Per block: 1 packed slot + 4 V matmuls.
  relu-h is one [128,1024] ACT op, out-relu one [128,1024] DVE op
  (PSUM->SBUF, the only engines that can read PSUM). Outputs are stored
  bf16 in a DMA-native packed layout and unscrambled on host.
"""

import math
import os

import ml_dtypes
import numpy as np

import concourse.bacc as bacc
import concourse.bass as bass
import concourse.mybir as mybir
import concourse.tile as tile
from concourse.bass_utils import run_bass_kernel_spmd

# ---- static module configuration (mirrors the torch source) ----
T = 9            # base types ("experts")
P_MAX = 4
D = 256
N_TYPES = 14
N_SRC = 5
NCORES = 8
BLOCK = 512      # edges per block (one PSUM bank per 128-out-dims half)
GRAN = 1         # run granularity (expert segments padded to per-core exact)

BASE_MAP = np.array([0, 0, 0, 1, 1, 1, 2, 2, 3, 4, 5, 6, 7, 8], dtype=np.int32)
PCOUNT = np.array([2, 2, 1, 1, 1, 1, 3, 2, 4], dtype=np.int32)
SCALES = np.ones((T, P_MAX), dtype=np.float32)
SCALES[0, :2] = [1.0, 1e-06]      # nmos  m, w
SCALES[1, :2] = [1.0, 1e-06]      # pmos  m, w
SCALES[2, 0] = 1.0                # balun rout
SCALES[3, 0] = 1000.0             # resistor r
SCALES[4, 0] = 1e-12              # capacitor c
SCALES[5, 0] = 1e-09              # inductor l
SCALES[6, :3] = [1.0, 1.0, 1.0]   # vsource dc, mag, phase
SCALES[7, :2] = [0.001, 0.001]    # isource dc, mag
SCALES[8, :4] = [1.0, 1.0, 1e9, 1.0]  # port dbm, dc, freq, num

# xu strip layout (replicated at partition offsets 0/32/64/96):
#   rows 0-3: scaled params, row 4: ones (valid), rows 5-18: type one-hot,
#   rows 19-23: source one-hot, rows 24-31: zero
K_L1 = 5                   # x rows + ones
K_G = 24                   # rows 0..23 (x rows are zero in the G weights)
STRIPS = (0, 32, 64, 96)   # (L1 h0, L1 h1, G g0, G g1)

_F32 = mybir.dt.float32
_BF16 = mybir.dt.bfloat16
_WARM_BURST = int(os.environ.get("EDGEENC_WARM_BURST", "10"))

_PROGRAM_CACHE: dict = {}
LAST_RESULT = None  # BassKernelResults of the most recent run (for test harness)


def _layout(base_ids: np.ndarray):
    """Per-expert per-core segment sizes (multiples of GRAN), identical on
    every core so one program serves all 8."""
    n_t = np.bincount(base_ids, minlength=T)
    m_t = np.zeros(T, dtype=np.int64)
    for t in range(T):
        if n_t[t] > 0:
            per_core = math.ceil(n_t[t] / NCORES)
            m_t[t] = math.ceil(per_core / GRAN) * GRAN
    L0 = int(m_t.sum())
    L = math.ceil(L0 / BLOCK) * BLOCK
    # fold the tail pad into the last present expert's segment
    last = int(np.nonzero(m_t)[0][-1])
    m_t[last] += L - L0
    return n_t, m_t, L


def _build_order(base_ids: np.ndarray, n_t, m_t, L) -> np.ndarray:
    """ORD[c, j] = global edge index at per-core slot j (or -1 = pad)."""
    ORD = np.full((NCORES, L), -1, dtype=np.int64)
    off = 0
    for t in range(T):
        if m_t[t] == 0:
            continue
        seg = int(m_t[t])
        idx = np.nonzero(base_ids == t)[0]
        arr = np.full(NCORES * seg, -1, dtype=np.int64)
        arr[: idx.shape[0]] = idx
        ORD[:, off : off + seg] = arr.reshape(NCORES, seg)
        off += seg
    return ORD


def _host_inputs(type_ids, source_ids, params, ORD):
    """XU[c] = [32, L] bf16: x rows, ones, type one-hot, src one-hot, zeros."""
    base_ids = BASE_MAP[type_ids]
    scales = SCALES[base_ids]                                  # [E,4]
    validp = np.arange(P_MAX)[None, :] < PCOUNT[base_ids][:, None]
    x = np.where(validp, params.astype(np.float32) / scales, 0.0).astype(np.float32)

    L = ORD.shape[1]
    XU = np.zeros((NCORES, 32, L), dtype=np.float32)
    valid = ORD >= 0
    ids = ORD[valid]
    tmp = np.zeros((NCORES, L, P_MAX), dtype=np.float32)
    tmp[valid] = x[ids]
    XU[:, 0:P_MAX, :] = tmp.transpose(0, 2, 1)
    XU[:, P_MAX, :] = valid
    ci, co = np.nonzero(valid)
    XU[ci, 5 + type_ids[ids], co] = 1.0
    XU[ci, 19 + source_ids[ids], co] = 1.0
    return XU.astype(ml_dtypes.bfloat16)


def _host_weights(type_embed, source_embed, W1, b1, W2, b2, Wf, bf):
    f = np.float32
    W1 = W1.astype(f); b1 = b1.astype(f); W2 = W2.astype(np.float64)
    b2 = b2.astype(f); Wf = Wf.astype(f); bf = bf.astype(f)
    type_embed = type_embed.astype(f); source_embed = source_embed.astype(f)

    Wft, Wfs, Wfp = Wf[:D], Wf[D : 2 * D], Wf[2 * D :]
    # V[t] = W2[t] @ Wf_pv (f64), fusing layer 2 with the final projection.
    V = (W2 @ Wfp.astype(np.float64)).astype(f)                 # [9,256,256]
    gt = type_embed @ Wft                                       # [14,256]
    gs = source_embed @ Wfs                                     # [5,256]
    gc = b2 @ Wfp + bf[None, :]                                 # [9,256]

    # W4 [128, T*512]: per expert t the four strip lhsT blocks
    #   [h0 | h1 | g0 | g1], each 128 cols.
    W4 = np.zeros((128, T * D * 2), dtype=f)
    VR = np.zeros((128, T * D * 2), dtype=f)
    for t in range(T):
        c = t * 512
        for h in range(2):
            W4[0:4, c + h * 128 : c + (h + 1) * 128] = W1[t][:, h * 128 : (h + 1) * 128]
            W4[4, c + h * 128 : c + (h + 1) * 128] = b1[t][h * 128 : (h + 1) * 128]
        for g in range(2):
            cg = c + 256 + g * 128
            W4[64 + 4, cg : cg + 128] = gc[t][g * 128 : (g + 1) * 128]
            W4[64 + 5 : 64 + 19, cg : cg + 128] = gt[:, g * 128 : (g + 1) * 128]
            W4[64 + 19 : 64 + 24, cg : cg + 128] = gs[:, g * 128 : (g + 1) * 128]
            # replicate for the 96-strip (g1 reads partitions 96..119)
            W4[96 + 4 : 96 + 24, cg : cg + 128] = W4[64 + 4 : 64 + 24, cg : cg + 128]
        # V chunks (h, g): rows = h dims, cols = out dims
        for h in range(2):
            for g in range(2):
                VR[:, c + h * 256 + g * 128 : c + h * 256 + (g + 1) * 128] = (
                    V[t][h * 128 : (h + 1) * 128, g * 128 : (g + 1) * 128])
    # strips 32/96 read their own partition rows; copy h1/g1 blocks there
    for t in range(T):
        c = t * 512
        W4[32:37, c + 128 : c + 256] = W4[0:5, c + 128 : c + 256]
        W4[0:5, c + 128 : c + 256] = 0.0
        W4[64 + 4 : 64 + 24, c + 384 : c + 512] = 0.0
    return W4.astype(ml_dtypes.bfloat16), VR.astype(ml_dtypes.bfloat16)


def _block_runs(m_t, L):
    """Per block: list of (c0, c1, expert) with cols relative to the block."""
    bounds = []
    off = 0
    for t in range(T):
        if m_t[t]:
            bounds.append((off, off + int(m_t[t]), t))
            off += int(m_t[t])
    if off < L:  # tail pad rides with the last expert
        bounds[-1] = (bounds[-1][0], L, bounds[-1][2])
    NB = L // BLOCK
    runs = [[] for _ in range(NB)]
    for (s0, s1, t) in bounds:
        b0, b1 = s0 // BLOCK, (s1 - 1) // BLOCK
        for b in range(b0, b1 + 1):
            c0 = max(s0 - b * BLOCK, 0)
            c1 = min(s1 - b * BLOCK, BLOCK)
            runs[b].append((c0, c1, t))
    return runs


def _build_program(m_t: tuple, L: int):
    """One compiled SPMD program for the given segment layout.

    Software-pipelined one block deep: emit slot(b+1) before V(b) so the
    PE never waits on the relu-h chain. PSUM: h pool 2x[128,512] (2
    banks), out pool 3x[128,1024] (6 banks) — the 3-deep out rotation
    keeps the G(b) -> V(b) -> DVE(b) -> G(b+3) buffer-reuse cycle off
    the critical path.
    """
    key = (m_t, L, _WARM_BURST)
    if key in _PROGRAM_CACHE:
        return _PROGRAM_CACHE[key]

    NB = L // BLOCK
    NSB = (NB + 1) // 2
    runs = _block_runs(np.asarray(m_t, dtype=np.int64), L)

    nc = bacc.Bacc("TRN2", target_bir_lowering=False, debug=False,
                   num_devices=NCORES)
    xu_d = nc.dram_tensor("xu", [32, L], _BF16, kind="ExternalInput")
    xu0_d = nc.dram_tensor("xu0", [128, 2 * BLOCK], _BF16, kind="ExternalInput")
    w4_d = nc.dram_tensor("w4", [128, T * 512], _BF16, kind="ExternalInput")
    vr_d = nc.dram_tensor("vr", [128, T * 512], _BF16, kind="ExternalInput")
    out_d = nc.dram_tensor("out", [128, NB * 1024], _BF16, kind="ExternalOutput")

    RELU = mybir.ActivationFunctionType.Relu

    with tile.TileContext(nc) as tc:
        with (
            tc.tile_pool(name="wts", bufs=1) as wts,
            tc.tile_pool(name="inp", bufs=3) as inp,
            tc.tile_pool(name="hsb", bufs=2) as hsbp,
            tc.tile_pool(name="osb", bufs=2) as osbp,
            tc.tile_pool(name="hps", bufs=3, space=bass.MemorySpace.PSUM) as hps,
            tc.tile_pool(name="ops", bufs=5, space=bass.MemorySpace.PSUM) as ops,
        ):
            w4 = wts.tile([128, T * 512], _BF16)
            vr = wts.tile([128, T * 512], _BF16)

            # prime the ACT table (Relu) before the first real activation
            prime = wts.tile([1, 8], _BF16)
            nc.gpsimd.memset(prime[:], 0.0)
            nc.scalar.activation(prime[0:1, 0:4], prime[0:1, 4:8], RELU)

            # bf16 warm-up burst: raise the PE HAM clock gate while the
            # first input/weight DMAs land
            if _WARM_BURST:
                wmw = wts.tile([128, 128], _BF16)
                wma = wts.tile([128, BLOCK], _BF16)
                nc.gpsimd.memset(wmw[:], 0.0)
                nc.gpsimd.memset(wma[:], 0.0)
                wmp = hps.tile([128, BLOCK], _F32, name="warmps", tag="h")
                for i in range(_WARM_BURST):
                    nc.tensor.matmul(wmp[:], wmw[:], wma[:],
                                     start=True, stop=True)

            emitted_w = set()

            def emit_weights(b):
                for (c0, c1, t) in runs[b]:
                    if t not in emitted_w:
                        emitted_w.add(t)
                        nc.sync.dma_start(w4[:, t * 512 : (t + 1) * 512],
                                          w4_d.ap()[:, t * 512 : (t + 1) * 512])
                        nc.sync.dma_start(vr[:, t * 512 : (t + 1) * 512],
                                          vr_d.ap()[:, t * 512 : (t + 1) * 512])

            xuts = {}

            def emit_input_sb(sb):
                if sb >= NSB:
                    return
                sbw = min(2 * BLOCK, L - sb * 1024)
                xut = inp.tile([128, 2 * BLOCK], _BF16, name=f"xu{sb}", tag="xu")
                if sb == 0:
                    # host pre-replicated: one DMA, no SBUF->SBUF chain at start
                    nc.gpsimd.dma_start(xut[:, 0:sbw], xu0_d.ap()[:, 0:sbw])
                else:
                    nc.gpsimd.dma_start(xut[0:32, 0:sbw],
                                        xu_d.ap()[:, sb * 1024 : sb * 1024 + sbw])
                    nc.gpsimd.dma_start(xut[32:64, 0:sbw], xut[0:32, 0:sbw])
                    nc.gpsimd.dma_start(xut[64:128, 0:sbw], xut[0:64, 0:sbw])
                xuts[sb] = xut

            hpt = {}   # (b, h) -> [128,512] psum tile
            hst = {}   # b -> [128,1024] sbuf bf16 relu(h)
            opt = {}   # (b, g) -> [128,512] psum out accumulator
            ost = {}   # sb -> [128,2048] sbuf bf16 out staging

            def emit_l1(b):
                """L1 h0/h1 on row strips 0/32 + relu-h ACT ops."""
                if b >= NB:
                    return
                emit_weights(b)
                if b % 2 == 0:
                    emit_input_sb(b // 2 + 1)   # prefetch next superblock
                xut = xuts[b // 2]
                off = (b % 2) * BLOCK
                hpt[(b, 0)] = hps.tile([128, BLOCK], _F32, name=f"h{b}_0", tag="h")
                hpt[(b, 1)] = hps.tile([128, BLOCK], _F32, name=f"h{b}_1", tag="h")
                for wi in range(2):
                    s = STRIPS[wi]
                    for (c0, c1, t) in runs[b]:
                        nc.tensor.matmul(
                            hpt[(b, wi)][:, c0:c1],
                            w4[s : s + K_L1, t * 512 + wi * 128
                               : t * 512 + wi * 128 + 128],
                            xut[s : s + K_L1, off + c0 : off + c1],
                            start=True, stop=True,
                            tile_position=(s, 0),
                        )
                hs = hsbp.tile([128, 1024], _BF16, name=f"hs{b}", tag="hs")
                nc.scalar.activation(hs[:, 0:BLOCK], hpt[(b, 0)][:], RELU)
                nc.scalar.activation(hs[:, BLOCK:1024], hpt[(b, 1)][:], RELU)
                hst[b] = hs

            def emit_v(b):
                """V accumulation; first chunk per bank opens the group
                (o-buf reuse lands here, ~2 blocks of slack)."""
                opt[(b, 0)] = ops.tile([128, BLOCK], _F32, name=f"o{b}_0", tag="o")
                opt[(b, 1)] = ops.tile([128, BLOCK], _F32, name=f"o{b}_1", tag="o")
                started = set()
                for h in range(2):
                    for g in range(2):
                        for (c0, c1, t) in runs[b]:
                            vcol = t * 512 + h * 256 + g * 128
                            start = g not in started
                            started.add(g)
                            nc.tensor.matmul(
                                opt[(b, g)][:, c0:c1],
                                vr[:, vcol : vcol + 128],
                                hst[b][:, h * BLOCK + c0 : h * BLOCK + c1],
                                start=start, stop=False,
                            )

            def emit_g_and_out(b):
                """G on strips 64/96 (concurrent with the next emit_l1's
                strips 0/32), closing each bank; then DVE out-relu + DMA."""
                xut = xuts[b // 2]
                off = (b % 2) * BLOCK
                nr = len(runs[b])
                for wi in (2, 3):
                    s = STRIPS[wi]
                    g = wi - 2
                    for i, (c0, c1, t) in enumerate(runs[b]):
                        nc.tensor.matmul(
                            opt[(b, g)][:, c0:c1],
                            w4[s : s + K_G, t * 512 + wi * 128
                               : t * 512 + wi * 128 + 128],
                            xut[s : s + K_G, off + c0 : off + c1],
                            start=False, stop=(i == nr - 1),
                            tile_position=(s, 0),
                        )
                sb = b // 2
                if b % 2 == 0:
                    ost[sb] = osbp.tile([128, 2048], _BF16, name=f"os{sb}",
                                        tag="os")
                for g in range(2):
                    nc.vector.tensor_scalar_max(
                        ost[sb][:, (b % 2) * 1024 + g * BLOCK
                                : (b % 2) * 1024 + (g + 1) * BLOCK],
                        opt[(b, g)][:], 0.0)
                sbw = min(2 * BLOCK, L - sb * 1024)
                if b == 2 * sb + 1 or b == NB - 1:
                    nc.sync.dma_start(
                        out_d.ap()[:, sb * 2048 : sb * 2048 + 2 * sbw],
                        ost[sb][:, 0 : 2 * sbw])

            emit_input_sb(0)
            emit_l1(0)
            emit_l1(1)
            for b in range(NB):
                emit_v(b)
                emit_g_and_out(b)   # strips 64/96 ...
                emit_l1(b + 2)      # ... run concurrent with strips 0/32

    nc.compile()
    _PROGRAM_CACHE[key] = nc
    return nc


def kernel(type_ids, source_ids, params, type_embed, source_embed,
           W1, b1, W2, b2, Wf, bf):
    global LAST_RESULT
    type_ids = np.asarray(type_ids, dtype=np.int32)
    source_ids = np.asarray(source_ids, dtype=np.int32)
    params = np.asarray(params, dtype=np.float32)
    E = type_ids.shape[0]

    base_ids = BASE_MAP[type_ids]
    n_t, m_t, L = _layout(base_ids)
    ORD = _build_order(base_ids, n_t, m_t, L)
    XU = _host_inputs(type_ids, source_ids, params, ORD)
    W4, VR = _host_weights(
        np.asarray(type_embed), np.asarray(source_embed),
        np.asarray(W1), np.asarray(b1), np.asarray(W2), np.asarray(b2),
        np.asarray(Wf), np.asarray(bf))

    nc = _build_program(tuple(int(v) for v in m_t), L)

    sbw0 = min(1024, L)
    in_maps = []
    for c in range(NCORES):
        xu0 = np.zeros((128, 1024), dtype=ml_dtypes.bfloat16)
        xu0[:, :sbw0] = np.tile(XU[c][:, :sbw0], (4, 1))
        in_maps.append({"xu": np.ascontiguousarray(XU[c]), "xu0": xu0,
                        "w4": W4, "vr": VR})

    trace = bool(int(os.environ.get("EDGEENC_TRACE", "0")))
    res = run_bass_kernel_spmd(nc, in_maps, core_ids=list(range(NCORES)),
                               trace=trace)
    LAST_RESULT = res

    NB = L // BLOCK
    full = np.zeros((E, D), dtype=np.float32)
    for c in range(NCORES):
        oc = res.results[c]["out"]                     # [128, NB*1024] bf16
        oc = np.asarray(oc)
        if oc.dtype != np.float32:
            oc = oc.astype(np.float32)
        # cols: [block b][g half][512 edges] -> [D, L]
        oc = oc.reshape(128, NB, 2, BLOCK)             # p, b, g, e
        oc = oc.transpose(2, 0, 1, 3).reshape(D, L)    # d = g*128+p
        sel = ORD[c] >= 0
        full[ORD[c][sel]] = np.ascontiguousarray(oc[:, sel].T)
    return full


# revision 19
# speedup vs baseline: 1.3164x; 1.1957x over previous
"""Trainium2 Bass kernel for nn_EdgeEncoder (moe_routing).

Strategy
--------
Each of E edges is routed to 1 of 9 expert MLPs (4 -> 256 -> 256), then
  out = relu(concat([type_embed[tid], source_embed[sid], pv]) @ Wf + bf).

Host (numpy, cheap O(E) work):
  * scale/mask params, group edge indices by expert (base type) at
    256-edge granularity, split evenly over 8 cores (identical layout on
    every core so one SPMD program serves all 8),
  * algebraic fusions: b1 rides a ones-row inside layer 1;
    V[t] = W2[t] @ Wf_pv fuses layer 2 with the final projection;
    G[t] = [const; type_embed@Wf_t; source_embed@Wf_s] turns the
    embedding gathers + all biases into one small matmul against the
    one-hot rows.

Device, all bf16 operands (fp32 PSUM accumulate; rel-err gate is 2e-2):
  The per-512-edge block needs h = relu(W1e.T @ x1) (2 matmuls, K=5),
  the G part (2 matmuls, K=24) and V part (4 matmuls, K=128) of the
  output. The four small-K matmuls are packed into ONE matmul slot via
  tile_position row-strips (0/32/64/96): the host ships a [32, L] input
  (x+ones+one-hots), the device replicates it to all four 32-partition
  strips with SBUF->SBUF DMA, and the four strip matmuls run
  concurrently in the PE array. Per block: 1 packed slot + 4 V matmuls.
  relu-h is one [128,1024] ACT op, out-relu one [128,1024] DVE op
  (PSUM->SBUF, the only engines that can read PSUM). Outputs are stored
  bf16 in a DMA-native packed layout and unscrambled on host.
"""

import math
import os

import ml_dtypes
import numpy as np

import concourse.bacc as bacc
import concourse.bass as bass
import concourse.mybir as mybir
import concourse.tile as tile
from concourse.bass_utils import run_bass_kernel_spmd

# ---- static module configuration (mirrors the torch source) ----
T = 9            # base types ("experts")
P_MAX = 4
D = 256
N_TYPES = 14
N_SRC = 5
NCORES = 8
BLOCK = 512      # edges per block (one PSUM bank per 128-out-dims half)
GRAN = 1         # run granularity (expert segments padded to per-core exact)

BASE_MAP = np.array([0, 0, 0, 1, 1, 1, 2, 2, 3, 4, 5, 6, 7, 8], dtype=np.int32)
PCOUNT = np.array([2, 2, 1, 1, 1, 1, 3, 2, 4], dtype=np.int32)
SCALES = np.ones((T, P_MAX), dtype=np.float32)
SCALES[0, :2] = [1.0, 1e-06]      # nmos  m, w
SCALES[1, :2] = [1.0, 1e-06]      # pmos  m, w
SCALES[2, 0] = 1.0                # balun rout
SCALES[3, 0] = 1000.0             # resistor r
SCALES[4, 0] = 1e-12              # capacitor c
SCALES[5, 0] = 1e-09              # inductor l
SCALES[6, :3] = [1.0, 1.0, 1.0]   # vsource dc, mag, phase
SCALES[7, :2] = [0.001, 0.001]    # isource dc, mag
SCALES[8, :4] = [1.0, 1.0, 1e9, 1.0]  # port dbm, dc, freq, num

# xu strip layout (replicated at partition offsets 0/32/64/96):
#   rows 0-3: scaled params, row 4: ones (valid), rows 5-18: type one-hot,
#   rows 19-23: source one-hot, rows 24-31: zero
K_L1 = 5                   # x rows + ones
K_G = 24                   # rows 0..23 (x rows are zero in the G weights)
STRIPS = (0, 32, 64, 96)   # (L1 h0, L1 h1, G g0, G g1)

_F32 = mybir.dt.float32
_BF16 = mybir.dt.bfloat16
_WARM_BURST = int(os.environ.get("EDGEENC_WARM_BURST", "10"))

_PROGRAM_CACHE: dict = {}
LAST_RESULT = None  # BassKernelResults of the most recent run (for test harness)


def _layout(base_ids: np.ndarray):
    """Per-expert per-core segment sizes (multiples of GRAN), identical on
    every core so one program serves all 8."""
    n_t = np.bincount(base_ids, minlength=T)
    m_t = np.zeros(T, dtype=np.int64)
    for t in range(T):
        if n_t[t] > 0:
            per_core = math.ceil(n_t[t] / NCORES)
            m_t[t] = math.ceil(per_core / GRAN) * GRAN
    L0 = int(m_t.sum())
    L = math.ceil(L0 / BLOCK) * BLOCK
    # fold the tail pad into the last present expert's segment
    last = int(np.nonzero(m_t)[0][-1])
    m_t[last] += L - L0
    return n_t, m_t, L


def _build_order(base_ids: np.ndarray, n_t, m_t, L) -> np.ndarray:
    """ORD[c, j] = global edge index at per-core slot j (or -1 = pad)."""
    ORD = np.full((NCORES, L), -1, dtype=np.int64)
    off = 0
    for t in range(T):
        if m_t[t] == 0:
            continue
        seg = int(m_t[t])
        idx = np.nonzero(base_ids == t)[0]
        arr = np.full(NCORES * seg, -1, dtype=np.int64)
        arr[: idx.shape[0]] = idx
        ORD[:, off : off + seg] = arr.reshape(NCORES, seg)
        off += seg
    return ORD


def _host_inputs(type_ids, source_ids, params, ORD):
    """XU[c] = [32, L] bf16: x rows, ones, type one-hot, src one-hot, zeros."""
    base_ids = BASE_MAP[type_ids]
    scales = SCALES[base_ids]                                  # [E,4]
    validp = np.arange(P_MAX)[None, :] < PCOUNT[base_ids][:, None]
    x = np.where(validp, params.astype(np.float32) / scales, 0.0).astype(np.float32)

    L = ORD.shape[1]
    XU = np.zeros((NCORES, 32, L), dtype=np.float32)
    valid = ORD >= 0
    ids = ORD[valid]
    tmp = np.zeros((NCORES, L, P_MAX), dtype=np.float32)
    tmp[valid] = x[ids]
    XU[:, 0:P_MAX, :] = tmp.transpose(0, 2, 1)
    XU[:, P_MAX, :] = valid
    ci, co = np.nonzero(valid)
    XU[ci, 5 + type_ids[ids], co] = 1.0
    XU[ci, 19 + source_ids[ids], co] = 1.0
    return XU.astype(ml_dtypes.bfloat16)


def _host_weights(type_embed, source_embed, W1, b1, W2, b2, Wf, bf):
    f = np.float32
    W1 = W1.astype(f); b1 = b1.astype(f); W2 = W2.astype(np.float64)
    b2 = b2.astype(f); Wf = Wf.astype(f); bf = bf.astype(f)
    type_embed = type_embed.astype(f); source_embed = source_embed.astype(f)

    Wft, Wfs, Wfp = Wf[:D], Wf[D : 2 * D], Wf[2 * D :]
    # V[t] = W2[t] @ Wf_pv (f64), fusing layer 2 with the final projection.
    V = (W2 @ Wfp.astype(np.float64)).astype(f)                 # [9,256,256]
    gt = type_embed @ Wft                                       # [14,256]
    gs = source_embed @ Wfs                                     # [5,256]
    gc = b2 @ Wfp + bf[None, :]                                 # [9,256]

    # W4 [128, T*512]: per expert t the four strip lhsT blocks
    #   [h0 | h1 | g0 | g1], each 128 cols.
    W4 = np.zeros((128, T * D * 2), dtype=f)
    VR = np.zeros((128, T * D * 2), dtype=f)
    for t in range(T):
        c = t * 512
        for h in range(2):
            W4[0:4, c + h * 128 : c + (h + 1) * 128] = W1[t][:, h * 128 : (h + 1) * 128]
            W4[4, c + h * 128 : c + (h + 1) * 128] = b1[t][h * 128 : (h + 1) * 128]
        for g in range(2):
            cg = c + 256 + g * 128
            W4[64 + 4, cg : cg + 128] = gc[t][g * 128 : (g + 1) * 128]
            W4[64 + 5 : 64 + 19, cg : cg + 128] = gt[:, g * 128 : (g + 1) * 128]
            W4[64 + 19 : 64 + 24, cg : cg + 128] = gs[:, g * 128 : (g + 1) * 128]
            # replicate for the 96-strip (g1 reads partitions 96..119)
            W4[96 + 4 : 96 + 24, cg : cg + 128] = W4[64 + 4 : 64 + 24, cg : cg + 128]
        # V chunks (h, g): rows = h dims, cols = out dims
        for h in range(2):
            for g in range(2):
                VR[:, c + h * 256 + g * 128 : c + h * 256 + (g + 1) * 128] = (
                    V[t][h * 128 : (h + 1) * 128, g * 128 : (g + 1) * 128])
    # strips 32/96 read their own partition rows; copy h1/g1 blocks there
    for t in range(T):
        c = t * 512
        W4[32:37, c + 128 : c + 256] = W4[0:5, c + 128 : c + 256]
        W4[0:5, c + 128 : c + 256] = 0.0
        W4[64 + 4 : 64 + 24, c + 384 : c + 512] = 0.0
    return W4.astype(ml_dtypes.bfloat16), VR.astype(ml_dtypes.bfloat16)


def _block_runs(m_t, L):
    """Per block: list of (c0, c1, expert) with cols relative to the block."""
    bounds = []
    off = 0
    for t in range(T):
        if m_t[t]:
            bounds.append((off, off + int(m_t[t]), t))
            off += int(m_t[t])
    if off < L:  # tail pad rides with the last expert
        bounds[-1] = (bounds[-1][0], L, bounds[-1][2])
    NB = L // BLOCK
    runs = [[] for _ in range(NB)]
    for (s0, s1, t) in bounds:
        b0, b1 = s0 // BLOCK, (s1 - 1) // BLOCK
        for b in range(b0, b1 + 1):
            c0 = max(s0 - b * BLOCK, 0)
            c1 = min(s1 - b * BLOCK, BLOCK)
            runs[b].append((c0, c1, t))
    return runs


def _build_program(m_t: tuple, L: int):
    """One compiled SPMD program for the given segment layout.

    Software-pipelined one block deep: emit slot(b+1) before V(b) so the
    PE never waits on the relu-h chain. PSUM: h pool 2x[128,512] (2
    banks), out pool 3x[128,1024] (6 banks) — the 3-deep out rotation
    keeps the G(b) -> V(b) -> DVE(b) -> G(b+3) buffer-reuse cycle off
    the critical path.
    """
    key = (m_t, L, _WARM_BURST)
    if key in _PROGRAM_CACHE:
        return _PROGRAM_CACHE[key]

    NB = L // BLOCK
    NSB = (NB + 1) // 2
    runs = _block_runs(np.asarray(m_t, dtype=np.int64), L)

    nc = bacc.Bacc("TRN2", target_bir_lowering=False, debug=False,
                   num_devices=NCORES)
    xu_d = nc.dram_tensor("xu", [128, L], _BF16, kind="ExternalInput")
    w4_d = nc.dram_tensor("w4", [128, T * 512], _BF16, kind="ExternalInput")
    vr_d = nc.dram_tensor("vr", [128, T * 512], _BF16, kind="ExternalInput")
    out_d = nc.dram_tensor("out", [128, NB * 1024], _BF16, kind="ExternalOutput")

    RELU = mybir.ActivationFunctionType.Relu

    with tile.TileContext(nc) as tc:
        with (
            tc.tile_pool(name="wts", bufs=1) as wts,
            tc.tile_pool(name="inp", bufs=3) as inp,
            tc.tile_pool(name="hsb", bufs=2) as hsbp,
            tc.tile_pool(name="osb", bufs=2) as osbp,
            tc.tile_pool(name="hps", bufs=3, space=bass.MemorySpace.PSUM) as hps,
            tc.tile_pool(name="ops", bufs=5, space=bass.MemorySpace.PSUM) as ops,
        ):
            w4 = wts.tile([128, T * 512], _BF16)
            vr = wts.tile([128, T * 512], _BF16)

            # prime the ACT table (Relu) before the first real activation
            prime = wts.tile([1, 8], _BF16)
            nc.gpsimd.memset(prime[:], 0.0)
            nc.scalar.activation(prime[0:1, 0:4], prime[0:1, 4:8], RELU)

            # bf16 warm-up burst: raise the PE HAM clock gate while the
            # first input/weight DMAs land
            if _WARM_BURST:
                wmw = wts.tile([128, 128], _BF16)
                wma = wts.tile([128, BLOCK], _BF16)
                nc.gpsimd.memset(wmw[:], 0.0)
                nc.gpsimd.memset(wma[:], 0.0)
                wmp = hps.tile([128, BLOCK], _F32, name="warmps", tag="h")
                for i in range(_WARM_BURST):
                    nc.tensor.matmul(wmp[:], wmw[:], wma[:],
                                     start=True, stop=True)

            emitted_w = set()

            def emit_weights(b):
                for (c0, c1, t) in runs[b]:
                    if t not in emitted_w:
                        emitted_w.add(t)
                        nc.gpsimd.dma_start(w4[:, t * 512 : (t + 1) * 512],
                                            w4_d.ap()[:, t * 512 : (t + 1) * 512])
                        nc.gpsimd.dma_start(vr[:, t * 512 : (t + 1) * 512],
                                            vr_d.ap()[:, t * 512 : (t + 1) * 512])

            xuts = {}

            def emit_input_sb(sb):
                if sb >= NSB:
                    return
                # host pre-replicates all 4 strips: one DMA, no SBUF chain
                sbw = min(2 * BLOCK, L - sb * 1024)
                xut = inp.tile([128, 2 * BLOCK], _BF16, name=f"xu{sb}", tag="xu")
                nc.gpsimd.dma_start(xut[:, 0:sbw],
                                    xu_d.ap()[:, sb * 1024 : sb * 1024 + sbw])
                xuts[sb] = xut

            hpt = {}   # (b, h) -> [128,512] psum tile
            hst = {}   # b -> [128,1024] sbuf bf16 relu(h)
            opt = {}   # (b, g) -> [128,512] psum out accumulator
            ost = {}   # sb -> [128,2048] sbuf bf16 out staging

            def emit_l1(b):
                """L1 h0/h1 on row strips 0/32 + relu-h ACT ops."""
                if b >= NB:
                    return
                emit_weights(b)
                if b % 2 == 0:
                    emit_input_sb(b // 2 + 1)   # prefetch next superblock
                xut = xuts[b // 2]
                off = (b % 2) * BLOCK
                hpt[(b, 0)] = hps.tile([128, BLOCK], _F32, name=f"h{b}_0", tag="h")
                hpt[(b, 1)] = hps.tile([128, BLOCK], _F32, name=f"h{b}_1", tag="h")
                for wi in range(2):
                    s = STRIPS[wi]
                    for (c0, c1, t) in runs[b]:
                        nc.tensor.matmul(
                            hpt[(b, wi)][:, c0:c1],
                            w4[s : s + K_L1, t * 512 + wi * 128
                               : t * 512 + wi * 128 + 128],
                            xut[s : s + K_L1, off + c0 : off + c1],
                            start=True, stop=True,
                            tile_position=(s, 0),
                        )
                hs = hsbp.tile([128, 1024], _BF16, name=f"hs{b}", tag="hs")
                nc.scalar.activation(hs[:, 0:BLOCK], hpt[(b, 0)][:], RELU)
                nc.scalar.activation(hs[:, BLOCK:1024], hpt[(b, 1)][:], RELU)
                hst[b] = hs

            def emit_v(b):
                """V accumulation; first chunk per bank opens the group
                (o-buf reuse lands here, ~2 blocks of slack)."""
                opt[(b, 0)] = ops.tile([128, BLOCK], _F32, name=f"o{b}_0", tag="o")
                opt[(b, 1)] = ops.tile([128, BLOCK], _F32, name=f"o{b}_1", tag="o")
                started = set()
                for h in range(2):
                    for g in range(2):
                        for (c0, c1, t) in runs[b]:
                            vcol = t * 512 + h * 256 + g * 128
                            start = g not in started
                            started.add(g)
                            nc.tensor.matmul(
                                opt[(b, g)][:, c0:c1],
                                vr[:, vcol : vcol + 128],
                                hst[b][:, h * BLOCK + c0 : h * BLOCK + c1],
                                start=start, stop=False,
                            )

            def emit_g_and_out(b):
                """G on strips 64/96 (concurrent with the next emit_l1's
                strips 0/32), closing each bank; then DVE out-relu + DMA."""
                xut = xuts[b // 2]
                off = (b % 2) * BLOCK
                nr = len(runs[b])
                for wi in (2, 3):
                    s = STRIPS[wi]
                    g = wi - 2
                    for i, (c0, c1, t) in enumerate(runs[b]):
                        nc.tensor.matmul(
                            opt[(b, g)][:, c0:c1],
                            w4[s : s + K_G, t * 512 + wi * 128
                               : t * 512 + wi * 128 + 128],
                            xut[s : s + K_G, off + c0 : off + c1],
                            start=False, stop=(i == nr - 1),
                            tile_position=(s, 0),
                        )
                sb = b // 2
                if b % 2 == 0:
                    ost[sb] = osbp.tile([128, 2048], _BF16, name=f"os{sb}",
                                        tag="os")
                for g in range(2):
                    nc.vector.tensor_scalar_max(
                        ost[sb][:, (b % 2) * 1024 + g * BLOCK
                                : (b % 2) * 1024 + (g + 1) * BLOCK],
                        opt[(b, g)][:], 0.0)
                sbw = min(2 * BLOCK, L - sb * 1024)
                if b == 2 * sb + 1 or b == NB - 1:
                    nc.sync.dma_start(
                        out_d.ap()[:, sb * 2048 : sb * 2048 + 2 * sbw],
                        ost[sb][:, 0 : 2 * sbw])

            emit_input_sb(0)
            emit_l1(0)
            emit_l1(1)
            for b in range(NB):
                emit_v(b)
                emit_g_and_out(b)   # strips 64/96 ...
                emit_l1(b + 2)      # ... run concurrent with strips 0/32

    nc.compile()
    _PROGRAM_CACHE[key] = nc
    return nc


def kernel(type_ids, source_ids, params, type_embed, source_embed,
           W1, b1, W2, b2, Wf, bf):
    global LAST_RESULT
    type_ids = np.asarray(type_ids, dtype=np.int32)
    source_ids = np.asarray(source_ids, dtype=np.int32)
    params = np.asarray(params, dtype=np.float32)
    E = type_ids.shape[0]

    base_ids = BASE_MAP[type_ids]
    n_t, m_t, L = _layout(base_ids)
    ORD = _build_order(base_ids, n_t, m_t, L)
    XU = _host_inputs(type_ids, source_ids, params, ORD)
    W4, VR = _host_weights(
        np.asarray(type_embed), np.asarray(source_embed),
        np.asarray(W1), np.asarray(b1), np.asarray(W2), np.asarray(b2),
        np.asarray(Wf), np.asarray(bf))

    nc = _build_program(tuple(int(v) for v in m_t), L)

    in_maps = []
    for c in range(NCORES):
        xu4 = np.ascontiguousarray(np.tile(XU[c], (4, 1)))   # [128, L]
        in_maps.append({"xu": xu4, "w4": W4, "vr": VR})

    trace = bool(int(os.environ.get("EDGEENC_TRACE", "0")))
    res = run_bass_kernel_spmd(nc, in_maps, core_ids=list(range(NCORES)),
                               trace=trace)
    LAST_RESULT = res

    NB = L // BLOCK
    full = np.zeros((E, D), dtype=np.float32)
    for c in range(NCORES):
        oc = res.results[c]["out"]                     # [128, NB*1024] bf16
        oc = np.asarray(oc)
        if oc.dtype != np.float32:
            oc = oc.astype(np.float32)
        # cols: [block b][g half][512 edges] -> [D, L]
        oc = oc.reshape(128, NB, 2, BLOCK)             # p, b, g, e
        oc = oc.transpose(2, 0, 1, 3).reshape(D, L)    # d = g*128+p
        sel = ORD[c] >= 0
        full[ORD[c][sel]] = np.ascontiguousarray(oc[:, sel].T)
    return full


# revision 25
# speedup vs baseline: 1.3860x; 1.0529x over previous
"""Trainium2 Bass kernel for nn_EdgeEncoder (moe_routing).

Strategy
--------
Each of E edges is routed to 1 of 9 expert MLPs (4 -> 256 -> 256), then
  out = relu(concat([type_embed[tid], source_embed[sid], pv]) @ Wf + bf).

Host (numpy, cheap O(E) work):
  * scale/mask params, group edge indices by expert (base type) at
    256-edge granularity, split evenly over 8 cores (identical layout on
    every core so one SPMD program serves all 8),
  * algebraic fusions: b1 rides a ones-row inside layer 1;
    V[t] = W2[t] @ Wf_pv fuses layer 2 with the final projection;
    G[t] = [const; type_embed@Wf_t; source_embed@Wf_s] turns the
    embedding gathers + all biases into one small matmul against the
    one-hot rows.

Device, all bf16 operands (fp32 PSUM accumulate; rel-err gate is 2e-2):
  The per-512-edge block needs h = relu(W1e.T @ x1) (2 matmuls, K=5),
  the G part (2 matmuls, K=24) and V part (4 matmuls, K=128) of the
  output. The four small-K matmuls are packed into ONE matmul slot via
  tile_position row-strips (0/32/64/96): the host ships a [32, L] input
  (x+ones+one-hots), the device replicates it to all four 32-partition
  strips with SBUF->SBUF DMA, and the four strip matmuls run
  concurrently in the PE array. Per block: 1 packed slot + 4 V matmuls.
  relu-h is one [128,1024] ACT op, out-relu one [128,1024] DVE op
  (PSUM->SBUF, the only engines that can read PSUM). Outputs are stored
  bf16 in a DMA-native packed layout and unscrambled on host.
"""

import math
import os

import ml_dtypes
import numpy as np

import concourse.bacc as bacc
import concourse.bass as bass
import concourse.mybir as mybir
import concourse.tile as tile
from concourse.bass_utils import run_bass_kernel_spmd

# ---- static module configuration (mirrors the torch source) ----
T = 9            # base types ("experts")
P_MAX = 4
D = 256
N_TYPES = 14
N_SRC = 5
NCORES = 8
BLOCK = 512      # edges per block (one PSUM bank per 128-out-dims half)
GRAN = 1         # run granularity (expert segments padded to per-core exact)

BASE_MAP = np.array([0, 0, 0, 1, 1, 1, 2, 2, 3, 4, 5, 6, 7, 8], dtype=np.int32)
PCOUNT = np.array([2, 2, 1, 1, 1, 1, 3, 2, 4], dtype=np.int32)
SCALES = np.ones((T, P_MAX), dtype=np.float32)
SCALES[0, :2] = [1.0, 1e-06]      # nmos  m, w
SCALES[1, :2] = [1.0, 1e-06]      # pmos  m, w
SCALES[2, 0] = 1.0                # balun rout
SCALES[3, 0] = 1000.0             # resistor r
SCALES[4, 0] = 1e-12              # capacitor c
SCALES[5, 0] = 1e-09              # inductor l
SCALES[6, :3] = [1.0, 1.0, 1.0]   # vsource dc, mag, phase
SCALES[7, :2] = [0.001, 0.001]    # isource dc, mag
SCALES[8, :4] = [1.0, 1.0, 1e9, 1.0]  # port dbm, dc, freq, num

# xu strip layout (replicated at partition offsets 0/32/64/96):
#   rows 0-3: scaled params, row 4: ones (valid), rows 5-18: type one-hot,
#   rows 19-23: source one-hot, rows 24-31: zero
K_L1 = 5                   # x rows + ones
K_G = 24                   # rows 0..23 (x rows are zero in the G weights)
STRIPS = (0, 32, 64, 96)   # (L1 h0, L1 h1, G g0, G g1)

_F32 = mybir.dt.float32
_BF16 = mybir.dt.bfloat16
_WARM_BURST = int(os.environ.get("EDGEENC_WARM_BURST", "6"))

_PROGRAM_CACHE: dict = {}
LAST_RESULT = None  # BassKernelResults of the most recent run (for test harness)


def _layout(base_ids: np.ndarray):
    """Per-expert per-core segment sizes (multiples of GRAN), identical on
    every core so one program serves all 8."""
    n_t = np.bincount(base_ids, minlength=T)
    m_t = np.zeros(T, dtype=np.int64)
    for t in range(T):
        if n_t[t] > 0:
            per_core = math.ceil(n_t[t] / NCORES)
            m_t[t] = math.ceil(per_core / GRAN) * GRAN
    L0 = int(m_t.sum())
    L = math.ceil(L0 / BLOCK) * BLOCK
    # fold the tail pad into the last present expert's segment
    last = int(np.nonzero(m_t)[0][-1])
    m_t[last] += L - L0
    return n_t, m_t, L


def _build_order(base_ids: np.ndarray, n_t, m_t, L) -> np.ndarray:
    """ORD[c, j] = global edge index at per-core slot j (or -1 = pad)."""
    ORD = np.full((NCORES, L), -1, dtype=np.int64)
    off = 0
    for t in range(T):
        if m_t[t] == 0:
            continue
        seg = int(m_t[t])
        idx = np.nonzero(base_ids == t)[0]
        arr = np.full(NCORES * seg, -1, dtype=np.int64)
        arr[: idx.shape[0]] = idx
        ORD[:, off : off + seg] = arr.reshape(NCORES, seg)
        off += seg
    return ORD


def _host_inputs(type_ids, source_ids, params, ORD):
    """XU[c] = [32, L] bf16: x rows, ones, type one-hot, src one-hot, zeros."""
    base_ids = BASE_MAP[type_ids]
    scales = SCALES[base_ids]                                  # [E,4]
    validp = np.arange(P_MAX)[None, :] < PCOUNT[base_ids][:, None]
    x = np.where(validp, params.astype(np.float32) / scales, 0.0).astype(np.float32)

    L = ORD.shape[1]
    XU = np.zeros((NCORES, 32, L), dtype=np.float32)
    valid = ORD >= 0
    ids = ORD[valid]
    tmp = np.zeros((NCORES, L, P_MAX), dtype=np.float32)
    tmp[valid] = x[ids]
    XU[:, 0:P_MAX, :] = tmp.transpose(0, 2, 1)
    XU[:, P_MAX, :] = valid
    ci, co = np.nonzero(valid)
    XU[ci, 5 + type_ids[ids], co] = 1.0
    XU[ci, 19 + source_ids[ids], co] = 1.0
    return XU.astype(ml_dtypes.bfloat16)


def _host_weights(type_embed, source_embed, W1, b1, W2, b2, Wf, bf):
    f = np.float32
    W1 = W1.astype(f); b1 = b1.astype(f); W2 = W2.astype(np.float64)
    b2 = b2.astype(f); Wf = Wf.astype(f); bf = bf.astype(f)
    type_embed = type_embed.astype(f); source_embed = source_embed.astype(f)

    Wft, Wfs, Wfp = Wf[:D], Wf[D : 2 * D], Wf[2 * D :]
    # V[t] = W2[t] @ Wf_pv (f64), fusing layer 2 with the final projection.
    V = (W2 @ Wfp.astype(np.float64)).astype(f)                 # [9,256,256]
    gt = type_embed @ Wft                                       # [14,256]
    gs = source_embed @ Wfs                                     # [5,256]
    gc = b2 @ Wfp + bf[None, :]                                 # [9,256]

    # W4 [128, T*512]: per expert t the four strip lhsT blocks
    #   [h0 | h1 | g0 | g1], each 128 cols.
    W4 = np.zeros((128, T * D * 2), dtype=f)
    VR = np.zeros((128, T * D * 2), dtype=f)
    for t in range(T):
        c = t * 512
        for h in range(2):
            W4[0:4, c + h * 128 : c + (h + 1) * 128] = W1[t][:, h * 128 : (h + 1) * 128]
            W4[4, c + h * 128 : c + (h + 1) * 128] = b1[t][h * 128 : (h + 1) * 128]
        for g in range(2):
            cg = c + 256 + g * 128
            W4[64 + 4, cg : cg + 128] = gc[t][g * 128 : (g + 1) * 128]
            W4[64 + 5 : 64 + 19, cg : cg + 128] = gt[:, g * 128 : (g + 1) * 128]
            W4[64 + 19 : 64 + 24, cg : cg + 128] = gs[:, g * 128 : (g + 1) * 128]
            # replicate for the 96-strip (g1 reads partitions 96..119)
            W4[96 + 4 : 96 + 24, cg : cg + 128] = W4[64 + 4 : 64 + 24, cg : cg + 128]
        # V chunks (h, g): rows = h dims, cols = out dims
        for h in range(2):
            for g in range(2):
                VR[:, c + h * 256 + g * 128 : c + h * 256 + (g + 1) * 128] = (
                    V[t][h * 128 : (h + 1) * 128, g * 128 : (g + 1) * 128])
    # strips 32/96 read their own partition rows; copy h1/g1 blocks there
    for t in range(T):
        c = t * 512
        W4[32:37, c + 128 : c + 256] = W4[0:5, c + 128 : c + 256]
        W4[0:5, c + 128 : c + 256] = 0.0
        W4[64 + 4 : 64 + 24, c + 384 : c + 512] = 0.0
    return W4.astype(ml_dtypes.bfloat16), VR.astype(ml_dtypes.bfloat16)


def _block_runs(m_t, L):
    """Per block: list of (c0, c1, expert) with cols relative to the block."""
    bounds = []
    off = 0
    for t in range(T):
        if m_t[t]:
            bounds.append((off, off + int(m_t[t]), t))
            off += int(m_t[t])
    if off < L:  # tail pad rides with the last expert
        bounds[-1] = (bounds[-1][0], L, bounds[-1][2])
    NB = L // BLOCK
    runs = [[] for _ in range(NB)]
    for (s0, s1, t) in bounds:
        b0, b1 = s0 // BLOCK, (s1 - 1) // BLOCK
        for b in range(b0, b1 + 1):
            c0 = max(s0 - b * BLOCK, 0)
            c1 = min(s1 - b * BLOCK, BLOCK)
            runs[b].append((c0, c1, t))
    return runs


def _build_program(m_t: tuple, L: int):
    """One compiled SPMD program for the given segment layout.

    Software-pipelined one block deep: emit slot(b+1) before V(b) so the
    PE never waits on the relu-h chain. PSUM: h pool 2x[128,512] (2
    banks), out pool 3x[128,1024] (6 banks) — the 3-deep out rotation
    keeps the G(b) -> V(b) -> DVE(b) -> G(b+3) buffer-reuse cycle off
    the critical path.
    """
    key = (m_t, L, _WARM_BURST)
    if key in _PROGRAM_CACHE:
        return _PROGRAM_CACHE[key]

    NB = L // BLOCK
    NSB = (NB + 1) // 2
    runs = _block_runs(np.asarray(m_t, dtype=np.int64), L)

    nc = bacc.Bacc("TRN2", target_bir_lowering=False, debug=False,
                   num_devices=NCORES)
    xu_d = nc.dram_tensor("xu", [128, L], _BF16, kind="ExternalInput")
    w4_d = nc.dram_tensor("w4", [128, T * 512], _BF16, kind="ExternalInput")
    vr_d = nc.dram_tensor("vr", [128, T * 512], _BF16, kind="ExternalInput")
    out_d = nc.dram_tensor("out", [128, NB * 1024], _BF16, kind="ExternalOutput")

    RELU = mybir.ActivationFunctionType.Relu

    with tile.TileContext(nc) as tc:
        with (
            tc.tile_pool(name="wts", bufs=1) as wts,
            tc.tile_pool(name="inp", bufs=3) as inp,
            tc.tile_pool(name="hsb", bufs=2) as hsbp,
            tc.tile_pool(name="osb", bufs=2) as osbp,
            tc.tile_pool(name="hps", bufs=1, space=bass.MemorySpace.PSUM) as hps,
            tc.tile_pool(name="ops", bufs=3, space=bass.MemorySpace.PSUM) as ops,
        ):
            w4 = wts.tile([128, T * 512], _BF16)
            vr = wts.tile([128, T * 512], _BF16)

            # prime the ACT table (Relu) before the first real activation
            prime = wts.tile([1, 8], _BF16)
            nc.gpsimd.memset(prime[:], 0.0)
            nc.scalar.activation(prime[0:1, 0:4], prime[0:1, 4:8], RELU)

            # bf16 warm-up burst: raise the PE HAM clock gate while the
            # first input/weight DMAs land
            if _WARM_BURST:
                wmw = wts.tile([128, 128], _BF16)
                wma = wts.tile([128, BLOCK], _BF16)
                nc.gpsimd.memset(wmw[:], 0.0)
                nc.gpsimd.memset(wma[:], 0.0)
                wmp = ops.tile([128, 1024], _F32, name="warmps", tag="o")
                for i in range(_WARM_BURST):
                    nc.tensor.matmul(wmp[:, 0:BLOCK], wmw[:], wma[:],
                                     start=True, stop=True)

            emitted_w = set()

            def emit_weights(b):
                for (c0, c1, t) in runs[b]:
                    if t not in emitted_w:
                        emitted_w.add(t)
                        nc.gpsimd.dma_start(w4[:, t * 512 : (t + 1) * 512],
                                            w4_d.ap()[:, t * 512 : (t + 1) * 512])
                        nc.gpsimd.dma_start(vr[:, t * 512 : (t + 1) * 512],
                                            vr_d.ap()[:, t * 512 : (t + 1) * 512])

            xuts = {}

            def emit_input_sb(sb):
                if sb >= NSB:
                    return
                # host pre-replicates all 4 strips: one DMA, no SBUF chain
                sbw = min(2 * BLOCK, L - sb * 1024)
                xut = inp.tile([128, 2 * BLOCK], _BF16, name=f"xu{sb}", tag="xu")
                nc.gpsimd.dma_start(xut[:, 0:sbw],
                                    xu_d.ap()[:, sb * 1024 : sb * 1024 + sbw])
                xuts[sb] = xut

            hpt = {}   # (b, h) -> [128,512] psum tile
            hst = {}   # b -> [128,1024] sbuf bf16 relu(h)
            opt = {}   # (b, g) -> [128,512] psum out accumulator
            ost = {}   # sb -> [128,2048] sbuf bf16 out staging

            def emit_l1(b):
                """L1 h0/h1 on row strips 0/32 + relu-h ACT ops."""
                if b >= NB:
                    return
                emit_weights(b)
                if b % 2 == 0:
                    emit_input_sb(b // 2 + 1)   # prefetch next superblock
                xut = xuts[b // 2]
                off = (b % 2) * BLOCK
                hpt[b] = hps.tile([128, 1024], _F32, name=f"h{b}", tag="h")
                for wi in range(2):
                    s = STRIPS[wi]
                    for (c0, c1, t) in runs[b]:
                        nc.tensor.matmul(
                            hpt[b][:, wi * BLOCK + c0 : wi * BLOCK + c1],
                            w4[s : s + K_L1, t * 512 + wi * 128
                               : t * 512 + wi * 128 + 128],
                            xut[s : s + K_L1, off + c0 : off + c1],
                            start=True, stop=True,
                            tile_position=(s, 0),
                        )
                hs = hsbp.tile([128, 1024], _BF16, name=f"hs{b}", tag="hs")
                nc.scalar.activation(hs[:], hpt[b][:], RELU)
                hst[b] = hs

            def emit_v(b):
                """V accumulation; first chunk per bank opens the group
                (o-buf reuse lands here, ~3 blocks of slack)."""
                opt[b] = ops.tile([128, 1024], _F32, name=f"o{b}", tag="o")
                started = set()
                for h in range(2):
                    for g in range(2):
                        for (c0, c1, t) in runs[b]:
                            vcol = t * 512 + h * 256 + g * 128
                            start = g not in started
                            started.add(g)
                            nc.tensor.matmul(
                                opt[b][:, g * BLOCK + c0 : g * BLOCK + c1],
                                vr[:, vcol : vcol + 128],
                                hst[b][:, h * BLOCK + c0 : h * BLOCK + c1],
                                start=start, stop=False,
                            )

            def emit_g_and_out(b):
                """G on strips 64/96 (concurrent with the next emit_l1's
                strips 0/32), closing each bank; then DVE out-relu + DMA."""
                xut = xuts[b // 2]
                off = (b % 2) * BLOCK
                nr = len(runs[b])
                for wi in (2, 3):
                    s = STRIPS[wi]
                    g = wi - 2
                    for i, (c0, c1, t) in enumerate(runs[b]):
                        nc.tensor.matmul(
                            opt[b][:, g * BLOCK + c0 : g * BLOCK + c1],
                            w4[s : s + K_G, t * 512 + wi * 128
                               : t * 512 + wi * 128 + 128],
                            xut[s : s + K_G, off + c0 : off + c1],
                            start=False, stop=(i == nr - 1),
                            tile_position=(s, 0),
                        )
                sb = b // 2
                if b % 2 == 0:
                    ost[sb] = osbp.tile([128, 2048], _BF16, name=f"os{sb}",
                                        tag="os")
                nc.vector.tensor_scalar_max(
                    ost[sb][:, (b % 2) * 1024 : (b % 2 + 1) * 1024],
                    opt[b][:], 0.0)
                sbw = min(2 * BLOCK, L - sb * 1024)
                if b == 2 * sb + 1 or b == NB - 1:
                    nc.sync.dma_start(
                        out_d.ap()[:, sb * 2048 : sb * 2048 + 2 * sbw],
                        ost[sb][:, 0 : 2 * sbw])

            emit_input_sb(0)
            emit_l1(0)
            emit_l1(1)
            for b in range(NB):
                emit_v(b)
                emit_g_and_out(b)   # strips 64/96 ...
                emit_l1(b + 2)      # ... run concurrent with strips 0/32

    nc.compile()
    _PROGRAM_CACHE[key] = nc
    return nc


def kernel(type_ids, source_ids, params, type_embed, source_embed,
           W1, b1, W2, b2, Wf, bf):
    global LAST_RESULT
    type_ids = np.asarray(type_ids, dtype=np.int32)
    source_ids = np.asarray(source_ids, dtype=np.int32)
    params = np.asarray(params, dtype=np.float32)
    E = type_ids.shape[0]

    base_ids = BASE_MAP[type_ids]
    n_t, m_t, L = _layout(base_ids)
    ORD = _build_order(base_ids, n_t, m_t, L)
    XU = _host_inputs(type_ids, source_ids, params, ORD)
    W4, VR = _host_weights(
        np.asarray(type_embed), np.asarray(source_embed),
        np.asarray(W1), np.asarray(b1), np.asarray(W2), np.asarray(b2),
        np.asarray(Wf), np.asarray(bf))

    nc = _build_program(tuple(int(v) for v in m_t), L)

    in_maps = []
    for c in range(NCORES):
        xu4 = np.ascontiguousarray(np.tile(XU[c], (4, 1)))   # [128, L]
        in_maps.append({"xu": xu4, "w4": W4, "vr": VR})

    trace = bool(int(os.environ.get("EDGEENC_TRACE", "0")))
    res = run_bass_kernel_spmd(nc, in_maps, core_ids=list(range(NCORES)),
                               trace=trace)
    LAST_RESULT = res

    NB = L // BLOCK
    full = np.zeros((E, D), dtype=np.float32)
    for c in range(NCORES):
        oc = res.results[c]["out"]                     # [128, NB*1024] bf16
        oc = np.asarray(oc)
        if oc.dtype != np.float32:
            oc = oc.astype(np.float32)
        # cols: [block b][g half][512 edges] -> [D, L]
        oc = oc.reshape(128, NB, 2, BLOCK)             # p, b, g, e
        oc = oc.transpose(2, 0, 1, 3).reshape(D, L)    # d = g*128+p
        sel = ORD[c] >= 0
        full[ORD[c][sel]] = np.ascontiguousarray(oc[:, sel].T)
    return full


# revision 29
# speedup vs baseline: 1.3916x; 1.0041x over previous
"""Trainium2 Bass kernel for nn_EdgeEncoder (moe_routing).

Strategy
--------
Each of E edges is routed to 1 of 9 expert MLPs (4 -> 256 -> 256), then
  out = relu(concat([type_embed[tid], source_embed[sid], pv]) @ Wf + bf).

Host (numpy, cheap O(E) work):
  * scale/mask params, group edge indices by expert (base type) at
    256-edge granularity, split evenly over 8 cores (identical layout on
    every core so one SPMD program serves all 8),
  * algebraic fusions: b1 rides a ones-row inside layer 1;
    V[t] = W2[t] @ Wf_pv fuses layer 2 with the final projection;
    G[t] = [const; type_embed@Wf_t; source_embed@Wf_s] turns the
    embedding gathers + all biases into one small matmul against the
    one-hot rows.

Device, all bf16 operands (fp32 PSUM accumulate; rel-err gate is 2e-2):
  The per-512-edge block needs h = relu(W1e.T @ x1) (2 matmuls, K=5),
  the G part (2 matmuls, K=24) and V part (4 matmuls, K=128) of the
  output. The four small-K matmuls are packed into ONE matmul slot via
  tile_position row-strips (0/32/64/96): the host ships a [32, L] input
  (x+ones+one-hots), the device replicates it to all four 32-partition
  strips with SBUF->SBUF DMA, and the four strip matmuls run
  concurrently in the PE array. Per block: 1 packed slot + 4 V matmuls.
  relu-h is one [128,1024] ACT op, out-relu one [128,1024] DVE op
  (PSUM->SBUF, the only engines that can read PSUM). Outputs are stored
  bf16 in a DMA-native packed layout and unscrambled on host.
"""

import math
import os

import ml_dtypes
import numpy as np

import concourse.bacc as bacc
import concourse.bass as bass
import concourse.mybir as mybir
import concourse.tile as tile
from concourse.bass_utils import run_bass_kernel_spmd

# ---- static module configuration (mirrors the torch source) ----
T = 9            # base types ("experts")
P_MAX = 4
D = 256
N_TYPES = 14
N_SRC = 5
NCORES = 8
BLOCK = 512      # edges per block (one PSUM bank per 128-out-dims half)
GRAN = 1         # run granularity (expert segments padded to per-core exact)

BASE_MAP = np.array([0, 0, 0, 1, 1, 1, 2, 2, 3, 4, 5, 6, 7, 8], dtype=np.int32)
PCOUNT = np.array([2, 2, 1, 1, 1, 1, 3, 2, 4], dtype=np.int32)
SCALES = np.ones((T, P_MAX), dtype=np.float32)
SCALES[0, :2] = [1.0, 1e-06]      # nmos  m, w
SCALES[1, :2] = [1.0, 1e-06]      # pmos  m, w
SCALES[2, 0] = 1.0                # balun rout
SCALES[3, 0] = 1000.0             # resistor r
SCALES[4, 0] = 1e-12              # capacitor c
SCALES[5, 0] = 1e-09              # inductor l
SCALES[6, :3] = [1.0, 1.0, 1.0]   # vsource dc, mag, phase
SCALES[7, :2] = [0.001, 0.001]    # isource dc, mag
SCALES[8, :4] = [1.0, 1.0, 1e9, 1.0]  # port dbm, dc, freq, num

# xu strip layout (replicated at partition offsets 0/32/64/96):
#   rows 0-3: scaled params, row 4: ones (valid), rows 5-18: type one-hot,
#   rows 19-23: source one-hot, rows 24-31: zero
K_L1 = 5                   # x rows + ones
K_G = 24                   # rows 0..23 (x rows are zero in the G weights)
STRIPS = (0, 32, 64, 96)   # (L1 h0, L1 h1, G g0, G g1)

_F32 = mybir.dt.float32
_BF16 = mybir.dt.bfloat16
_WARM_BURST = int(os.environ.get("EDGEENC_WARM_BURST", "6"))

_PROGRAM_CACHE: dict = {}
LAST_RESULT = None  # BassKernelResults of the most recent run (for test harness)


def _layout(base_ids: np.ndarray):
    """Per-expert per-core segment sizes (multiples of GRAN), identical on
    every core so one program serves all 8."""
    n_t = np.bincount(base_ids, minlength=T)
    m_t = np.zeros(T, dtype=np.int64)
    for t in range(T):
        if n_t[t] > 0:
            per_core = math.ceil(n_t[t] / NCORES)
            m_t[t] = math.ceil(per_core / GRAN) * GRAN
    L0 = int(m_t.sum())
    L = math.ceil(L0 / BLOCK) * BLOCK
    # fold the tail pad into the last present expert's segment
    last = int(np.nonzero(m_t)[0][-1])
    m_t[last] += L - L0
    return n_t, m_t, L


def _build_order(base_ids: np.ndarray, n_t, m_t, L) -> np.ndarray:
    """ORD[c, j] = global edge index at per-core slot j (or -1 = pad)."""
    ORD = np.full((NCORES, L), -1, dtype=np.int64)
    off = 0
    for t in range(T):
        if m_t[t] == 0:
            continue
        seg = int(m_t[t])
        idx = np.nonzero(base_ids == t)[0]
        arr = np.full(NCORES * seg, -1, dtype=np.int64)
        arr[: idx.shape[0]] = idx
        ORD[:, off : off + seg] = arr.reshape(NCORES, seg)
        off += seg
    return ORD


def _host_inputs(type_ids, source_ids, params, ORD):
    """XU[c] = [32, L] bf16: x rows, ones, type one-hot, src one-hot, zeros."""
    base_ids = BASE_MAP[type_ids]
    scales = SCALES[base_ids]                                  # [E,4]
    validp = np.arange(P_MAX)[None, :] < PCOUNT[base_ids][:, None]
    x = np.where(validp, params.astype(np.float32) / scales, 0.0).astype(np.float32)

    L = ORD.shape[1]
    XU = np.zeros((NCORES, 32, L), dtype=np.float32)
    valid = ORD >= 0
    ids = ORD[valid]
    tmp = np.zeros((NCORES, L, P_MAX), dtype=np.float32)
    tmp[valid] = x[ids]
    XU[:, 0:P_MAX, :] = tmp.transpose(0, 2, 1)
    XU[:, P_MAX, :] = valid
    ci, co = np.nonzero(valid)
    XU[ci, 5 + type_ids[ids], co] = 1.0
    XU[ci, 19 + source_ids[ids], co] = 1.0
    return XU.astype(ml_dtypes.bfloat16)


def _host_weights(type_embed, source_embed, W1, b1, W2, b2, Wf, bf):
    f = np.float32
    W1 = W1.astype(f); b1 = b1.astype(f); W2 = W2.astype(np.float64)
    b2 = b2.astype(f); Wf = Wf.astype(f); bf = bf.astype(f)
    type_embed = type_embed.astype(f); source_embed = source_embed.astype(f)

    Wft, Wfs, Wfp = Wf[:D], Wf[D : 2 * D], Wf[2 * D :]
    # V[t] = W2[t] @ Wf_pv (f64), fusing layer 2 with the final projection.
    V = (W2 @ Wfp.astype(np.float64)).astype(f)                 # [9,256,256]
    gt = type_embed @ Wft                                       # [14,256]
    gs = source_embed @ Wfs                                     # [5,256]
    gc = b2 @ Wfp + bf[None, :]                                 # [9,256]

    # W4 [128, T*512]: per expert t the four strip lhsT blocks
    #   [h0 | h1 | g0 | g1], each 128 cols.
    W4 = np.zeros((128, T * D * 2), dtype=f)
    VR = np.zeros((128, T * D * 2), dtype=f)
    for t in range(T):
        c = t * 512
        for h in range(2):
            W4[0:4, c + h * 128 : c + (h + 1) * 128] = W1[t][:, h * 128 : (h + 1) * 128]
            W4[4, c + h * 128 : c + (h + 1) * 128] = b1[t][h * 128 : (h + 1) * 128]
        for g in range(2):
            cg = c + 256 + g * 128
            W4[64 + 4, cg : cg + 128] = gc[t][g * 128 : (g + 1) * 128]
            W4[64 + 5 : 64 + 19, cg : cg + 128] = gt[:, g * 128 : (g + 1) * 128]
            W4[64 + 19 : 64 + 24, cg : cg + 128] = gs[:, g * 128 : (g + 1) * 128]
            # replicate for the 96-strip (g1 reads partitions 96..119)
            W4[96 + 4 : 96 + 24, cg : cg + 128] = W4[64 + 4 : 64 + 24, cg : cg + 128]
        # V chunks (h, g): rows = h dims, cols = out dims
        for h in range(2):
            for g in range(2):
                VR[:, c + h * 256 + g * 128 : c + h * 256 + (g + 1) * 128] = (
                    V[t][h * 128 : (h + 1) * 128, g * 128 : (g + 1) * 128])
    # strips 32/96 read their own partition rows; copy h1/g1 blocks there
    for t in range(T):
        c = t * 512
        W4[32:37, c + 128 : c + 256] = W4[0:5, c + 128 : c + 256]
        W4[0:5, c + 128 : c + 256] = 0.0
        W4[64 + 4 : 64 + 24, c + 384 : c + 512] = 0.0
    return W4.astype(ml_dtypes.bfloat16), VR.astype(ml_dtypes.bfloat16)


def _block_runs(m_t, L):
    """Per block: list of (c0, c1, expert) with cols relative to the block."""
    bounds = []
    off = 0
    for t in range(T):
        if m_t[t]:
            bounds.append((off, off + int(m_t[t]), t))
            off += int(m_t[t])
    if off < L:  # tail pad rides with the last expert
        bounds[-1] = (bounds[-1][0], L, bounds[-1][2])
    NB = L // BLOCK
    runs = [[] for _ in range(NB)]
    for (s0, s1, t) in bounds:
        b0, b1 = s0 // BLOCK, (s1 - 1) // BLOCK
        for b in range(b0, b1 + 1):
            c0 = max(s0 - b * BLOCK, 0)
            c1 = min(s1 - b * BLOCK, BLOCK)
            runs[b].append((c0, c1, t))
    return runs


def _build_program(m_t: tuple, L: int):
    """One compiled SPMD program for the given segment layout.

    Software-pipelined one block deep: emit slot(b+1) before V(b) so the
    PE never waits on the relu-h chain. PSUM: h pool 2x[128,512] (2
    banks), out pool 3x[128,1024] (6 banks) — the 3-deep out rotation
    keeps the G(b) -> V(b) -> DVE(b) -> G(b+3) buffer-reuse cycle off
    the critical path.
    """
    key = (m_t, L, _WARM_BURST)
    if key in _PROGRAM_CACHE:
        return _PROGRAM_CACHE[key]

    NB = L // BLOCK
    NSB = (NB + 1) // 2
    runs = _block_runs(np.asarray(m_t, dtype=np.int64), L)

    nc = bacc.Bacc("TRN2", target_bir_lowering=False, debug=False,
                   num_devices=NCORES)
    xu_d = nc.dram_tensor("xu", [128, L], _BF16, kind="ExternalInput")
    w4_d = nc.dram_tensor("w4", [128, T * 512], _BF16, kind="ExternalInput")
    vr_d = nc.dram_tensor("vr", [128, T * 512], _BF16, kind="ExternalInput")
    out_d = nc.dram_tensor("out", [128, NB * 1024], _BF16, kind="ExternalOutput")

    RELU = mybir.ActivationFunctionType.Relu

    with tile.TileContext(nc) as tc:
        with (
            tc.tile_pool(name="wts", bufs=1) as wts,
            tc.tile_pool(name="inp", bufs=3) as inp,
            tc.tile_pool(name="hsb", bufs=2) as hsbp,
            tc.tile_pool(name="osb", bufs=2) as osbp,
            tc.tile_pool(name="hps", bufs=2, space=bass.MemorySpace.PSUM) as hps,
            tc.tile_pool(name="ops", bufs=4, space=bass.MemorySpace.PSUM) as ops,
        ):
            w4 = wts.tile([128, T * 512], _BF16)
            vr = wts.tile([128, T * 512], _BF16)

            # prime the ACT table (Relu) before the first real activation
            prime = wts.tile([1, 8], _BF16)
            nc.gpsimd.memset(prime[:], 0.0)
            nc.scalar.activation(prime[0:1, 0:4], prime[0:1, 4:8], RELU)

            # bf16 warm-up burst: raise the PE HAM clock gate while the
            # first input/weight DMAs land
            if _WARM_BURST:
                wmw = wts.tile([128, 128], _BF16)
                wma = wts.tile([128, BLOCK], _BF16)
                nc.gpsimd.memset(wmw[:], 0.0)
                nc.gpsimd.memset(wma[:], 0.0)
                wmp = ops.tile([128, BLOCK], _F32, name="warmps", tag="o")
                for i in range(_WARM_BURST):
                    nc.tensor.matmul(wmp[:], wmw[:], wma[:],
                                     start=True, stop=True)

            emitted_w = set()

            def emit_weights(b):
                for (c0, c1, t) in runs[b]:
                    if t not in emitted_w:
                        emitted_w.add(t)
                        nc.gpsimd.dma_start(w4[:, t * 512 : (t + 1) * 512],
                                            w4_d.ap()[:, t * 512 : (t + 1) * 512])
                        nc.gpsimd.dma_start(vr[:, t * 512 : (t + 1) * 512],
                                            vr_d.ap()[:, t * 512 : (t + 1) * 512])

            xuts = {}

            def emit_input_sb(sb):
                if sb >= NSB:
                    return
                # host pre-replicates all 4 strips: one DMA, no SBUF chain
                sbw = min(2 * BLOCK, L - sb * 1024)
                xut = inp.tile([128, 2 * BLOCK], _BF16, name=f"xu{sb}", tag="xu")
                nc.gpsimd.dma_start(xut[:, 0:sbw],
                                    xu_d.ap()[:, sb * 1024 : sb * 1024 + sbw])
                xuts[sb] = xut

            hpt = {}   # (b, h) -> [128,512] psum tile
            hst = {}   # b -> [128,1024] sbuf bf16 relu(h)
            opt = {}   # (b, g) -> [128,512] psum out accumulator
            ost = {}   # sb -> [128,2048] sbuf bf16 out staging

            def emit_l1(b):
                """L1 h0/h1 on row strips 0/32 + relu-h ACT ops."""
                if b >= NB:
                    return
                emit_weights(b)
                if b % 2 == 0:
                    emit_input_sb(b // 2 + 1)   # prefetch next superblock
                xut = xuts[b // 2]
                off = (b % 2) * BLOCK
                hpt[b] = hps.tile([128, 1024], _F32, name=f"h{b}", tag="h")
                for wi in range(2):
                    s = STRIPS[wi]
                    for (c0, c1, t) in runs[b]:
                        nc.tensor.matmul(
                            hpt[b][:, wi * BLOCK + c0 : wi * BLOCK + c1],
                            w4[s : s + K_L1, t * 512 + wi * 128
                               : t * 512 + wi * 128 + 128],
                            xut[s : s + K_L1, off + c0 : off + c1],
                            start=True, stop=True,
                            tile_position=(s, 0),
                        )
                hs = hsbp.tile([128, 1024], _BF16, name=f"hs{b}", tag="hs")
                nc.scalar.activation(hs[:], hpt[b][:], RELU)
                hst[b] = hs

            def emit_v(b):
                """V accumulation, g-major: bank g0 finishes first so its
                DVE op (and the o-buf reuse two blocks later) start early."""
                opt[(b, 0)] = ops.tile([128, BLOCK], _F32, name=f"o{b}_0", tag="o")
                opt[(b, 1)] = ops.tile([128, BLOCK], _F32, name=f"o{b}_1", tag="o")
                started = set()
                for g in range(2):
                    for h in range(2):
                        for (c0, c1, t) in runs[b]:
                            vcol = t * 512 + h * 256 + g * 128
                            start = g not in started
                            started.add(g)
                            nc.tensor.matmul(
                                opt[(b, g)][:, c0:c1],
                                vr[:, vcol : vcol + 128],
                                hst[b][:, h * BLOCK + c0 : h * BLOCK + c1],
                                start=start, stop=False,
                            )

            def emit_g_and_out(b):
                """G on strips 64/96 (concurrent with the next emit_l1's
                strips 0/32), closing each bank; then DVE out-relu + DMA."""
                xut = xuts[b // 2]
                off = (b % 2) * BLOCK
                nr = len(runs[b])
                for wi in (2, 3):
                    s = STRIPS[wi]
                    g = wi - 2
                    for i, (c0, c1, t) in enumerate(runs[b]):
                        nc.tensor.matmul(
                            opt[(b, g)][:, c0:c1],
                            w4[s : s + K_G, t * 512 + wi * 128
                               : t * 512 + wi * 128 + 128],
                            xut[s : s + K_G, off + c0 : off + c1],
                            start=False, stop=(i == nr - 1),
                            tile_position=(s, 0),
                        )
                sb = b // 2
                if b % 2 == 0:
                    ost[sb] = osbp.tile([128, 2048], _BF16, name=f"os{sb}",
                                        tag="os")
                for g in range(2):
                    nc.vector.tensor_scalar_max(
                        ost[sb][:, (b % 2) * 1024 + g * BLOCK
                                : (b % 2) * 1024 + (g + 1) * BLOCK],
                        opt[(b, g)][:], 0.0)
                sbw = min(2 * BLOCK, L - sb * 1024)
                if b == 2 * sb + 1 or b == NB - 1:
                    nc.sync.dma_start(
                        out_d.ap()[:, sb * 2048 : sb * 2048 + 2 * sbw],
                        ost[sb][:, 0 : 2 * sbw])

            emit_input_sb(0)
            emit_l1(0)
            emit_l1(1)
            for b in range(NB):
                emit_v(b)
                emit_g_and_out(b)   # strips 64/96 ...
                emit_l1(b + 2)      # ... run concurrent with strips 0/32

    nc.compile()
    _PROGRAM_CACHE[key] = nc
    return nc


def kernel(type_ids, source_ids, params, type_embed, source_embed,
           W1, b1, W2, b2, Wf, bf):
    global LAST_RESULT
    type_ids = np.asarray(type_ids, dtype=np.int32)
    source_ids = np.asarray(source_ids, dtype=np.int32)
    params = np.asarray(params, dtype=np.float32)
    E = type_ids.shape[0]

    base_ids = BASE_MAP[type_ids]
    n_t, m_t, L = _layout(base_ids)
    ORD = _build_order(base_ids, n_t, m_t, L)
    XU = _host_inputs(type_ids, source_ids, params, ORD)
    W4, VR = _host_weights(
        np.asarray(type_embed), np.asarray(source_embed),
        np.asarray(W1), np.asarray(b1), np.asarray(W2), np.asarray(b2),
        np.asarray(Wf), np.asarray(bf))

    nc = _build_program(tuple(int(v) for v in m_t), L)

    in_maps = []
    for c in range(NCORES):
        xu4 = np.ascontiguousarray(np.tile(XU[c], (4, 1)))   # [128, L]
        in_maps.append({"xu": xu4, "w4": W4, "vr": VR})

    trace = bool(int(os.environ.get("EDGEENC_TRACE", "0")))
    res = run_bass_kernel_spmd(nc, in_maps, core_ids=list(range(NCORES)),
                               trace=trace)
    LAST_RESULT = res

    NB = L // BLOCK
    full = np.zeros((E, D), dtype=np.float32)
    for c in range(NCORES):
        oc = res.results[c]["out"]                     # [128, NB*1024] bf16
        oc = np.asarray(oc)
        if oc.dtype != np.float32:
            oc = oc.astype(np.float32)
        # cols: [block b][g half][512 edges] -> [D, L]
        oc = oc.reshape(128, NB, 2, BLOCK)             # p, b, g, e
        oc = oc.transpose(2, 0, 1, 3).reshape(D, L)    # d = g*128+p
        sel = ORD[c] >= 0
        full[ORD[c][sel]] = np.ascontiguousarray(oc[:, sel].T)
    return full
